# revision 25
# baseline (speedup 1.0000x reference)
"""Fused int8 dequant -> causal mask -> softmax -> int8 requant on 8 TRN2 cores.

Problem: x_q [B=4, H=16, S=1024, S] int8, per-(head,row) scales sx/so [H*S] f32.
  out = int8(clip(round(softmax(causal_mask(x_q * sx)) / so), -128, 127))

Sharding: 2 heads per core (data parallel over 64 independent (b, h) planes;
grouping by head lets the 4 batches of one head share per-partition scale
vectors, so the exp runs as one instruction per (h, row-tile)).

Rows live on partitions; softmax runs along the free dim. For each (h, t)
row-tile of 128 rows, only cols [0, W=(t+1)*128) can be nonzero (causal), so
only those are moved (44% of DMA traffic saved; the untouched upper triangle
stays zero because output buffers are pre-zeroed by the runtime).

Host-side prep makes the device work trivial:
  - x is pre-masked (strict upper triangle zeroed), so the only masked
    cleanup left on device is the diagonal 128x128 block of each row-tile.
  - x/y use a packed per-(h,t) tile layout ([128, 4b * W] contiguous per
    block), so every DMA moves 128 descriptors of 4*W (up to 16 KiB) bytes
    instead of 512 x W-byte strided rows (the packet-rate bound at 542 B
    packets measured 188 GB/s; packed gets byte-bound ~330 GB/s).

Device pipeline per (h, t):
  1. one DMA in:  xt [128, 4W] int8
  2. ScalarE: et = Exp(sx[row] * xt) -> fp16. Row sums are the expensive
     part: every reduce-class DVE op measures 1x (1 elt/cyc/lane), while
     ACT's accum_out is free FD-wise but costs ~460ns/instr of overhead at
     the forced per-(b,t) granularity (8192 rows / 128 partitions = 64
     readouts). So sums are SPLIT: large tiles (t >= DVE_SUM_NT) use per-b
     exp+accum_out on ACT; small tiles use one batched exp plus per-b
     tensor_scalar+accum_out on DVE, balancing the two engines' busy time.
     Premasked x makes masked lanes contribute exp(0)=1, corrected by the
     compile-time constant (127 - p) before use.
  3. DVE: zero the masked (strict upper) part of each diagonal block of et
     (in-place fp16 tensor_tensor with a lower-tri 0/1 mask, 2x mode)
  4. smalls: r = 1/((sum - corr) * so)
  5. DVE per b: y = et_b * r -> int8 (runs in 2x mode). The HW f32->int8
     conversion is round-to-nearest-even WITH saturation (measured on DVE
     and ACT), which is exactly jnp's round+clip; masked positions 0*r -> 0.
  6. one DMA out: yt [128, 4W] int8
(fp16 et: element rounding gives measured end-to-end flip rate 4.7e-05 at
absmax 1 vs the f32 reference; sums accumulate in f32.)
"""

import contextlib
import ctypes
import os
import sys
import types
from contextlib import ExitStack

import numpy as np

import concourse.bacc as bacc
import concourse.bass as bass
import concourse.tile as tile
from concourse import mybir
from concourse.bass_utils import run_bass_kernel_spmd

B, H, S = 4, 16, 1024
NCORES = 8
HPC = H // NCORES  # heads per core
P = 128
NT = S // P  # row tiles per plane
AF = mybir.ActivationFunctionType
ALU = mybir.AluOpType

# packed block offsets: block (h, t) holds [P, B*W] int8, W = (t+1)*P
_BLK = [[None] * NT for _ in range(HPC)]
_off = 0
for _h in range(HPC):
    for _t in range(NT):
        _W = (_t + 1) * P
        _BLK[_h][_t] = (_off, _W)
        _off += P * B * _W
TOTAL = _off  # per-core packed bytes (4718592)

_AXON_SO = "/opt/axon/libaxon_pjrt.so"


def _ensure_ntff_hook():
    """This image's antenv lacks axon_hooks; provide it so trace=True works."""
    if "antenv.axon_hooks" in sys.modules:
        return
    import antenv

    mod = types.ModuleType("antenv.axon_hooks")
    state = {"hook": None}
    mod.set_axon_ntff_profile_hook = lambda h: state.__setitem__("hook", h)
    mod.get_axon_ntff_profile_hook = lambda: state["hook"]
    sys.modules["antenv.axon_hooks"] = mod
    antenv.axon_hooks = mod

    if not os.path.exists(_AXON_SO):
        return
    lib = ctypes.CDLL(_AXON_SO)
    if not hasattr(lib, "axon_start_nrt_profile"):
        return
    lib.axon_start_nrt_profile.argtypes = [ctypes.POINTER(ctypes.c_int64), ctypes.c_size_t]
    lib.axon_start_nrt_profile.restype = ctypes.c_int64
    lib.axon_stop_nrt_profile.argtypes = [ctypes.c_char_p]
    lib.axon_stop_nrt_profile.restype = ctypes.c_int64

    @contextlib.contextmanager
    def _hook(output_dir, device_ids):
        import jax

        jax.devices()
        if device_ids:
            ids = (ctypes.c_int64 * len(device_ids))(*device_ids)
            rc = lib.axon_start_nrt_profile(ids, len(device_ids))
        else:
            rc = lib.axon_start_nrt_profile(None, 0)
        if rc != 0:
            raise RuntimeError(f"axon_start_nrt_profile rc={rc}")
        try:
            yield
        finally:
            n = lib.axon_stop_nrt_profile(str(output_dir).encode())
            print(f"profile: {n} file(s) written to {output_dir}", file=sys.stderr)

    mod.set_axon_ntff_profile_hook(_hook)


_cached_nc = None


DVE_SUM_NT = 5  # tiles t < DVE_SUM_NT sum on DVE; the rest on ACT accum


def _build_bass(compile=True):
    nc = bacc.Bacc("TRN2", target_bir_lowering=False, debug=False,
                   num_devices=NCORES)
    x = nc.declare_dram_parameter("x", [TOTAL], mybir.dt.int8, isOutput=False)
    sx = nc.declare_dram_parameter("sx", [P, HPC * NT], mybir.dt.float32, isOutput=False)
    so = nc.declare_dram_parameter("so", [P, HPC * NT], mybir.dt.float32, isOutput=False)
    corr = nc.declare_dram_parameter("corr", [P, 1], mybir.dt.float32, isOutput=False)
    tri = nc.declare_dram_parameter("tri", [P, P], mybir.dt.float16, isOutput=False)
    y = nc.declare_dram_parameter("y", [TOTAL], mybir.dt.int8, isOutput=True)

    with ExitStack() as ctx:
        tc = ctx.enter_context(tile.TileContext(nc))
        singles = ctx.enter_context(tc.tile_pool(name="singles", bufs=1))
        xpool = ctx.enter_context(tc.tile_pool(name="xp", bufs=6))
        epool = ctx.enter_context(tc.tile_pool(name="ep", bufs=4))
        spool = ctx.enter_context(tc.tile_pool(name="sp", bufs=3))
        ypool = ctx.enter_context(tc.tile_pool(name="yp", bufs=6))
        smalls = ctx.enter_context(tc.tile_pool(name="sm", bufs=8))

        sxt = singles.tile([P, HPC * NT], mybir.dt.float32)
        nc.sync.dma_start(sxt[:], sx[:])
        sot = singles.tile([P, HPC * NT], mybir.dt.float32)
        nc.sync.dma_start(sot[:], so[:])
        corrt = singles.tile([P, 1], mybir.dt.float32)
        nc.sync.dma_start(corrt[:], corr[:])
        trit = singles.tile([P, P], mybir.dt.float16)
        nc.sync.dma_start(trit[:], tri[:])

        for h in range(HPC):
            for t in range(NT):
                off, W = _BLK[h][t]
                col = h * NT + t

                xt = xpool.tile([P, B * W], mybir.dt.int8, tag="xt")
                nc.sync.dma_start(
                    xt[:], x[off:off + P * B * W].rearrange("(p n) -> p n", p=P))

                et = epool.tile([P, B * W], mybir.dt.float16, tag="et")
                sums = smalls.tile([P, B], mybir.dt.float32, tag="sums")
                if t < DVE_SUM_NT:
                    # batched exp; per-b sums via DVE tensor_scalar accum
                    nc.scalar.activation(et[:], xt[:], AF.Exp, bias=0.0,
                                         scale=sxt[:, col:col + 1])
                    scr = spool.tile([P, DVE_SUM_NT * P * B], mybir.dt.float16,
                                     tag="scr")
                    for b in range(B):
                        nc.vector.tensor_scalar(
                            scr[:, b * W:(b + 1) * W], et[:, b * W:(b + 1) * W],
                            1.0, None, ALU.mult, ALU.add,
                            accum_out=sums[:, b:b + 1])
                else:
                    # per-b exp with free row sums from the ACT accumulator
                    for b in range(B):
                        nc.scalar.activation(et[:, b * W:(b + 1) * W],
                                             xt[:, b * W:(b + 1) * W],
                                             AF.Exp, bias=0.0,
                                             scale=sxt[:, col:col + 1],
                                             accum_out=sums[:, b:b + 1])

                # zero the masked (strict upper) part of the diagonal block so
                # the requant writes true zeros there; one strided TT covers
                # all 4 b's, with the tri mask broadcast via a stride-0 dim
                dz = t * P
                diag = bass.AP(tensor=et.tensor, offset=et.offset + dz,
                               ap=[et.ap[0], [W, B], [1, P]])
                trib = bass.AP(tensor=trit.tensor, offset=trit.offset,
                               ap=[trit.ap[0], [0, B], [1, P]])
                nc.vector.tensor_tensor(diag, diag, trib, ALU.mult)

                rt = smalls.tile([P, B], mybir.dt.float32, tag="rt")
                nc.vector.tensor_scalar(rt[:], sums[:], corrt[:], None, ALU.subtract)
                nc.vector.tensor_scalar(rt[:], rt[:], sot[:, col:col + 1], None, ALU.mult)
                nc.vector.reciprocal(rt[:], rt[:])

                yt = ypool.tile([P, B * W], mybir.dt.int8, tag="yt")
                for b in range(B):
                    nc.vector.tensor_scalar(yt[:, b * W:(b + 1) * W],
                                            et[:, b * W:(b + 1) * W],
                                            rt[:, b:b + 1], None, ALU.mult)

                nc.sync.dma_start(
                    y[off:off + P * B * W].rearrange("(p n) -> p n", p=P), yt[:])
    if compile:
        nc.compile()
    return nc


_tril_mask = None


def _host_prep(x_q, scale_x, scale_out):
    global _tril_mask
    x_q = np.asarray(x_q)
    assert x_q.dtype == np.int8, x_q.dtype
    scale_x = np.asarray(scale_x, dtype=np.float32).reshape(H, S)
    scale_out = np.asarray(scale_out, dtype=np.float32).reshape(H, S)

    if _tril_mask is None:
        _tril_mask = np.tril(np.ones((S, S), dtype=np.int8))
    x_pm = x_q * _tril_mask  # zero the strict upper triangle

    # [P, H, NT]: sxr[p, h, t] = scale_x[h, t*128 + p]
    sxr = scale_x.reshape(H, NT, P).transpose(2, 0, 1)
    sor = scale_out.reshape(H, NT, P).transpose(2, 0, 1)

    corr = (127 - np.arange(P)).astype(np.float32).reshape(P, 1)
    tri = np.tril(np.ones((P, P), dtype=np.float16))

    in_maps = []
    for c in range(NCORES):
        xc = np.empty(TOTAL, np.int8)
        for h in range(HPC):
            hg = c * HPC + h
            for t in range(NT):
                off, W = _BLK[h][t]
                # [B, P, W] -> [P, B, W] flattened
                blk = x_pm[:, hg, t * P:(t + 1) * P, 0:W].transpose(1, 0, 2)
                xc[off:off + P * B * W] = blk.reshape(-1)
        hs = slice(c * HPC, (c + 1) * HPC)
        sxc = np.ascontiguousarray(sxr[:, hs].reshape(P, HPC * NT))
        soc = np.ascontiguousarray(sor[:, hs].reshape(P, HPC * NT))
        in_maps.append({"x": xc, "sx": sxc, "so": soc, "corr": corr, "tri": tri})
    return in_maps


def _host_unpack(results):
    out = np.zeros((B, H, S, S), np.int8)
    for c in range(NCORES):
        yc = np.asarray(results[c]["y"])
        for h in range(HPC):
            hg = c * HPC + h
            for t in range(NT):
                off, W = _BLK[h][t]
                blk = yc[off:off + P * B * W].reshape(P, B, W).transpose(1, 0, 2)
                out[:, hg, t * P:(t + 1) * P, 0:W] = blk
    return out


def run(x_q, scale_x, scale_out, trace=False):
    global _cached_nc
    if trace:
        _ensure_ntff_hook()
    if _cached_nc is None:
        _cached_nc = _build_bass()
    in_maps = _host_prep(x_q, scale_x, scale_out)
    res = run_bass_kernel_spmd(_cached_nc, in_maps, core_ids=list(range(NCORES)),
                               trace=trace)
    return _host_unpack(res.results), res


def kernel(x_q, scale_x, scale_out):
    out, _ = run(x_q, scale_x, scale_out,
                 trace=bool(int(os.environ.get("KERNEL_TRACE", "0"))))
    return out
